# revision 36
# baseline (speedup 1.0000x reference)
"""ConvLSTM encoder + autoregressive decoder on 8 TRN2 NeuronCores.

Problem: B=8, T=12, H=W=128, C=1, F=64; fused-gate ConvLSTM (Keras order
i,f,g,o) for 12 steps, then 6 decoder steps:
    pred = sigmoid(conv3x3(h, w_out) + b_out)
    cur  = relu(conv1x1(pred, w_proj) + b_proj)

Sharding: pure data-parallel — core b computes batch element b. No
collectives.

Per-core dataflow (one batch element):
  * h lives in SBUF as bf16 in a zero-padded [ch, HP, WP] layout (HP=H+2),
    duplicated at two row shifts so that a 3x3 conv becomes 6 TensorE
    matmuls per 512-pixel chunk per 128-wide output-channel half:
      - "dup" tile: partitions 0-63  = hpad shifted +1 row (S1)
                    partitions 64-127 = hpad (S0)
        -> one matmul contracts K=128 = taps (0,dx) and (-1,dx) at once
           (3 "domino" matmuls), plus 2 "single" K=64 matmuls for taps
           (+1,-1), (+1,+1) reading partitions 0-63.
      - "hx" tile: partitions 0-63 = S1 copy, partitions 64-72 = 9 rows of
        host-im2col'ed input patches -> one K=73 matmul covers tap (+1,0)
        AND the whole 3x3x1->256 input conv.
  * PSUM [128, CH] accumulates z for a 2-gate half; ScalarE applies
    Sigmoid/Tanh (+bias) straight out of PSUM; VectorE does the gate
    products; c stays fp32 in SBUF.
  * Decoder: relu(w_proj*p + b_proj) is exactly linear in p on (0,1) when
    the biases don't flip its sign (true for this problem's zero biases),
    so steps 2..6 collapse to a 1-channel 3x3 conv, computed as 9 tiny
    [128,128] fp32 matmuls with banded row-shift matrices.
"""

import numpy as np
import ml_dtypes

import concourse.bass as bass
import concourse.bacc as bacc
import concourse.mybir as mybir
import concourse.tile as tile

F32 = mybir.dt.float32
BF16 = mybir.dt.bfloat16
FP8 = mybir.dt.float8e4
DRPM = mybir.MatmulPerfMode.DoubleRow
HDT = BF16          # dtype of h-state tiles + conv weights on device
HDT_NP = ml_dtypes.bfloat16
CDT = BF16          # dtype of the cell state c
WSC = 16.0          # fp8 weight pre-scale; z activations descale by 1/WSC
SIG = mybir.ActivationFunctionType.Sigmoid
TANH = mybir.ActivationFunctionType.Tanh
MULT = mybir.AluOpType.mult
ADD = mybir.AluOpType.add

TAPS = [(dy, dx) for dy in (-1, 0, 1) for dx in (-1, 0, 1)]

# full-problem geometry
B = 8
T = 12
H = W = 128
F = 64
PRED = 6


class Geo:
    def __init__(self, H, W, T, PRED, RPC=8, SUB=4):
        self.H, self.W, self.T, self.PRED = H, W, T, PRED
        self.HP, self.WP = H + 2, W + 2
        self.RPC = RPC              # output rows per outer chunk
        self.SUB = SUB              # output rows per matmul (N = SUB*W <= 512)
        assert H % RPC == 0 and RPC % SUB == 0
        self.NOC = H // RPC         # outer chunks
        self.NSUB = RPC // SUB      # matmul subchunks per outer chunk
        self.CH = RPC * W           # pixels per outer chunk
        self.N = SUB * W            # matmul moving size
        assert self.N <= 512 and self.CH * 4 <= 4096


def pack_host(G, kernel, rec_kernel, bias, w_out, b_out, w_proj, b_proj):
    """Host-side weight packing. All inputs are full-precision numpy."""
    kernel = np.asarray(kernel, np.float32)
    rec_kernel = np.asarray(rec_kernel, np.float32)
    bias = np.asarray(bias, np.float32)
    w_out = np.asarray(w_out, np.float32)
    b_out = np.asarray(b_out, np.float32)
    w_proj = np.asarray(w_proj, np.float32)
    b_proj = np.asarray(b_proj, np.float32)
    Fl = rec_kernel.shape[2]
    C4 = rec_kernel.shape[3]
    assert C4 == 4 * Fl
    # output-channel permutations: half0 = [f; i], half1 = [o; g]
    # (o in the lower half so sig_o is base-partition-aligned with tanh_c
    # for the h product — TensorTensor requires equal SBUF base partitions)
    perm = [
        np.concatenate([np.arange(Fl, 2 * Fl), np.arange(0, Fl)]),
        np.concatenate([np.arange(3 * Fl, 4 * Fl), np.arange(2 * Fl, 3 * Fl)]),
    ]

    # The g-gate gets a doubled pre-activation so tanh(g) comes from the
    # sigmoid shared with the o-gate: tanh(z) = 2*sig(2z)-1. h and c are
    # stored undoubled; tanh(c) uses the Tanh activation directly (same hw
    # act table as Sigmoid, so no table reload).
    #   g-out-cols  *= 2         (for rec, x, and bias alike)
    s_out = np.ones(C4, np.float32)
    s_out[2 * Fl : 3 * Fl] = 2.0
    rec_eff = rec_kernel * s_out
    kern_eff = kernel * s_out
    bias = bias * s_out

    def Wt(dy, dx):
        return rec_eff[dy + 1, dx + 1]  # (F, 4F)

    xk = kern_eff.reshape(9, C4)  # rows in TAPS order

    # fp8 DoubleRow recurrent weights, scaled by WS to sit in e4m3's normal
    # range (the z activations descale by 1/WS). One DoubleRow matmul per
    # stencil column dx: partition halves give taps (0,dx)/(-1,dx) via the
    # row-shifted dup packing, the DoubleRow k-group (row-stride) gives
    # (+1,dx) on the S1 half (S0 slot of group1 duplicates tap (0,dx) and
    # is zeroed).
    WS = WSC
    fp8 = ml_dtypes.float8_e4m3
    w_q = np.zeros((2, 3, 2 * Fl, 2, 2 * Fl), np.float32)
    w_combx = np.zeros((2, 10, 2 * Fl), np.float32)
    for h in range(2):
        p = perm[h]
        for i, dx in enumerate((-1, 0, 1)):
            w_q[h, i, 0:Fl, 0] = Wt(0, dx)[:, p] * WS
            w_q[h, i, Fl : 2 * Fl, 0] = Wt(-1, dx)[:, p] * WS
            w_q[h, i, 0:Fl, 1] = Wt(1, dx)[:, p] * WS
        w_combx[h, 0:9] = xk[:, p] * WS
        w_combx[h, 9] = bias[p] * WS   # bias rides the ones-plane

    # decoder first conv (M=1) from h
    wo = w_out[:, :, :, 0]  # (3,3,F)

    def Wo(dy, dx):
        return wo[dy + 1, dx + 1]  # (F,)

    p0_dom = np.zeros((3, 2 * Fl, 1), np.float32)
    p0_hdom = np.zeros((2 * Fl, 1), np.float32)
    p0_c11 = np.zeros((Fl, 1), np.float32)
    for i, dx in enumerate((-1, 0, 1)):
        p0_dom[i, :, 0] = np.concatenate([Wo(0, dx), Wo(-1, dx)])
    p0_hdom[:, 0] = np.concatenate([Wo(1, -1), Wo(1, 0)])
    p0_c11[:, 0] = Wo(1, 1)

    # collapse relu(w_proj*p + b_proj) to A*p + d on p in (0,1)
    wp = w_proj[0, 0, 0, :]  # (F,)
    lo = np.minimum(b_proj, wp + b_proj)
    hi = np.maximum(b_proj, wp + b_proj)
    pos = lo >= 0.0
    neg = hi <= 0.0
    if not np.all(pos | neg):
        raise NotImplementedError(
            "decoder relu is not linear on (0,1) for some channel; "
            "general path not implemented"
        )
    A = np.where(pos, wp, 0.0).astype(np.float32)
    d = np.where(pos, b_proj, 0.0).astype(np.float32)
    if np.any(d != 0.0):
        raise NotImplementedError("nonzero collapsed intercept not implemented")
    K2 = np.array(
        [wo[dy + 1, dx + 1] @ A for (dy, dx) in TAPS], np.float32
    )  # (9,) — this conv consumes pred directly
    c0 = float(b_out[0])

    Hh = G.H
    w_dec = np.zeros((9, Hh, Hh), np.float32)
    for k, (dy, dx) in enumerate(TAPS):
        w_dec[k] = K2[k] * np.eye(Hh, k=-dy, dtype=np.float32)

    bf = HDT_NP
    return {
        "w_q": w_q.reshape(2, 3, 2 * Fl, 2 * (2 * Fl)).astype(fp8),
        "w_combx": w_combx.astype(bf),
        "p0_dom": p0_dom.astype(bf),
        "p0_hdom": p0_hdom.astype(bf),
        "p0_c11": p0_c11.astype(bf),
        "w_dec": w_dec,
    }, float(b_out[0]), c0


def make_xcol(G, xb):
    """xb: (T, H, W) fp32 for one batch element -> (T, 10, HP, WP) bf16.

    Plane 9 is all-ones: it carries the gate bias through the input-conv
    matmul so the two z sigmoids can merge into one activation call."""
    Tn, HP, WP = G.T, G.HP, G.WP
    xpad = np.zeros((Tn, HP, WP), np.float32)
    xpad[:, 1 : G.H + 1, 1 : G.W + 1] = xb
    flat = xpad.reshape(Tn, HP * WP)
    out = np.zeros((Tn, 10, HP * WP), np.float32)
    n = HP * WP
    for k, (dy, dx) in enumerate(TAPS):
        off = dy * WP + dx
        slo, shi = max(0, off), n + min(0, off)
        dlo = max(0, -off)
        out[:, k, dlo : dlo + (shi - slo)] = flat[:, slo:shi]
    out[:, 9, :] = 1.0
    return out.astype(HDT_NP)


def build(G, b_out_f, c0_f, debug_state=False, phase="all", repeat_enc=1,
          sim_compat=False):
    """Build the Bass program (same for every core)."""
    nc = bacc.Bacc("TRN2", target_bir_lowering=False, debug=False)
    Fl = F
    HP, WP, CH, N, SUB, RPC = G.HP, G.WP, G.CH, G.N, G.SUB, G.RPC
    W = G.W

    xcol = nc.dram_tensor("xcol", [G.T, 10, HP * WP], HDT, kind="ExternalInput")
    d_wq = nc.dram_tensor(
        "w_q", [2, 3, 2 * Fl, 2 * (2 * Fl)], FP8, kind="ExternalInput"
    )
    d_wcombx = nc.dram_tensor("w_combx", [2, 10, 2 * Fl], HDT, kind="ExternalInput")
    d_p0dom = nc.dram_tensor("p0_dom", [3, 2 * Fl, 1], HDT, kind="ExternalInput")
    d_p0hdom = nc.dram_tensor("p0_hdom", [2 * Fl, 1], HDT, kind="ExternalInput")
    d_p0c11 = nc.dram_tensor("p0_c11", [Fl, 1], HDT, kind="ExternalInput")
    d_wdec = nc.dram_tensor("w_dec", [9, G.H, G.H], F32, kind="ExternalInput")

    out = nc.dram_tensor("out", [G.PRED, G.H * G.W], F32, kind="ExternalOutput")
    if debug_state:
        dbg_h = nc.dram_tensor("dbg_h", [F, G.HP, G.WP], F32, kind="ExternalOutput")
        dbg_h1 = nc.dram_tensor("dbg_h1", [F, G.HP, G.WP], F32, kind="ExternalOutput")
        dbg_c = nc.dram_tensor("dbg_c", [F, G.H * G.W], F32, kind="ExternalOutput")

    with tile.TileContext(nc) as tc:
        with (
            tc.tile_pool(name="persist", bufs=1) as pp,
            tc.tile_pool(name="dram", bufs=1, space="DRAM") as dp,
        ):
            # persistent state
            # bf16 h tiles — written every step (dup S0 is the h product's
            # landing spot and the cast source); S1/hx2 shifted copies are
            # only made on the final step, for the decoder.
            dup = pp.tile([128, HP, WP], HDT)   # [S1=h+1row; S0=h]
            hx2 = pp.tile([128, HP, WP], HDT)   # [h+1row-1col; h+1row]
            # fp8 recurrent tile: same [S1; S0] packing as dup. The three
            # DoubleRow matmuls per (sub, half) read it with a row-stride
            # k-group, covering all 9 taps.
            dupq = pp.tile([128, HP, WP], FP8)
            xpat = pp.tile([10, HP, WP], HDT)   # input patches + ones plane
            ct = pp.tile([Fl, G.H * G.W], CDT)  # cell state
            # zero-init: regions read but never written — padding borders.
            # (Interiors are fully written by phase_y(t) before being read.)
            nc.vector.memset(ct[:, :], 0.0)
            nc.vector.memset(dup[:, :, 0:1], 0.0)
            nc.vector.memset(dup[:, :, WP - 1 : WP], 0.0)
            nc.vector.memset(dup[Fl : 2 * Fl, 0:1, :], 0.0)
            nc.vector.memset(dup[0:Fl, G.H : G.H + 1, :], 0.0)
            nc.vector.memset(dupq[:, :, 0:1], 0.0)
            nc.vector.memset(dupq[:, :, WP - 1 : WP], 0.0)
            nc.vector.memset(dupq[Fl : 2 * Fl, 0:1, :], 0.0)
            nc.vector.memset(dupq[0:Fl, G.H : G.H + 1, :], 0.0)
            nc.vector.memset(hx2[:, G.H : G.H + 1, :], 0.0)
            nc.vector.memset(hx2[0:Fl, :, 1:2], 0.0)

            # weights
            wq = []      # wq[h][i]: [128, 2, 128] fp8 DoubleRow stationary
            wcombx = []  # [9, 2Fl] bf16 input-conv stationary
            for h in range(2):
                row = []
                for i in range(3):
                    t = pp.tile([2 * Fl, 2, 2 * Fl], FP8, tag=f"wq{h}{i}")
                    nc.sync.dma_start(
                        t[:, :, :],
                        d_wq[h, i].rearrange("p (a b) -> p a b", a=2),
                    )
                    row.append(t)
                wq.append(row)
                t = pp.tile([10, 2 * Fl], HDT, tag=f"wcombx{h}")
                nc.sync.dma_start(t[:, :], d_wcombx[h])
                wcombx.append(t)

            # ---------------- encoder ----------------
            if phase in ("all", "enc"):
              with (
                tc.tile_pool(name="ps", bufs=2, space="PSUM") as ps,
                tc.tile_pool(name="gs", bufs=3) as gs,
              ):
                from contextlib import nullcontext
                loop_cm = (
                    tc.For_i(0, repeat_enc, 1) if repeat_enc > 1 else nullcontext()
                )
                with loop_cm:
                  for t in range(G.T):
                    # stream this step's input patches into xpat,
                    # split per row-region so each DMA's WAR wait (vs the
                    # previous step's comb reads of that region) resolves early
                    for rg in range(G.NOC):
                        r0 = rg * RPC
                        r1 = HP if rg == G.NOC - 1 else (rg + 1) * RPC
                        nc.sync.dma_start(
                            xpat[:, r0:r1, :].rearrange("p a b -> p (a b)"),
                            xcol[t, :, r0 * WP : r1 * WP],
                        )

                    def dr_rhs(ys, dx):
                        # [K=128, 2, SUB, W] fp8: k-group dim strides one ROW,
                        # so group0 = taps (0,dx)/(-1,dx) via the S1/S0
                        # partition packing and group1 = (+1,dx) on S1 (the
                        # S0 slot of group1 duplicates (0,dx) — zero weights).
                        # Fine-grained (sub-row) group strides crash the PE —
                        # row-stride groups are HW-validated.
                        r = dupq[:, ys : ys + SUB, 1 + dx : 1 + dx + W].unsqueeze(1)
                        r.ap[1] = [WP, 2]
                        return r

                    # Phase 1: all matmuls of this step. Emitting every
                    # conv read before any h-write keeps the in-place h
                    # update race-free (chunk oc+1's dy=-1 tap reads the
                    # previous step's last row of chunk oc). Weight-major
                    # order within a (chunk, half) so the stationary matrix
                    # loads once per NSUB output windows.
                    pzs = []
                    for oc in range(G.NOC):
                        y0 = oc * RPC
                        pzm = ps.tile([128, 2 * CH], F32, tag="psz",
                                      name=f"psz_{t}_{oc}")
                        pz = [pzm[:, 0:CH], pzm[:, CH : 2 * CH]]
                        pzs.append(pzm)
                        for h in range(2):
                            mm = []
                            if t > 0:
                                for i, dx in enumerate((-1, 0, 1)):
                                    if sim_compat:
                                        # two plain fp8 matmuls, same math
                                        for g in range(2):
                                            mm.append(
                                                (
                                                    wq[h][i][:, g, :],
                                                    lambda ys, dx=dx, g=g: dupq[
                                                        :,
                                                        ys + g : ys + g + SUB,
                                                        1 + dx : 1 + dx + W,
                                                    ],
                                                    None,
                                                )
                                            )
                                    else:
                                        mm.append(
                                            (
                                                wq[h][i][:, :, :],
                                                lambda ys, dx=dx: dr_rhs(ys, dx),
                                                DRPM,
                                            )
                                        )
                            mm.append(
                                (
                                    wcombx[h][:, :],
                                    lambda ys: xpat[
                                        :, ys + 1 : ys + 1 + SUB, 1 : 1 + W
                                    ],
                                    None,
                                )
                            )
                            for i, (lhsT, rhs_at, pm) in enumerate(mm):
                                for s in range(G.NSUB):
                                    ys = y0 + s * SUB
                                    nc.tensor.matmul(
                                        pzm[:, h * CH + s * N : h * CH + (s + 1) * N],
                                        lhsT,
                                        rhs_at(ys),
                                        start=(i == 0),
                                        stop=(i == len(mm) - 1),
                                        perf_mode=pm,
                                    )

                    # Phase 2: gate math, software-pipelined with a 2-chunk
                    # skew so every cross-engine wait is pre-satisfied when it
                    # reaches the head of its (in-order) engine queue — a
                    # blocking semaphore wake costs ~8us on this part.
                    SKEW = 2
                    stash = {}

                    def phase_x(oc):
                        px0 = oc * CH
                        pzm = pzs[oc]
                        sg = gs.tile([128, 2 * CH], HDT, tag="sg",
                                     name=f"sg_{t}_{oc}")
                        yp = gs.tile([128, CH], HDT, tag="yp", name=f"yp_{t}_{oc}")
                        # one sigmoid covers all four gates: [f;i | o;2g]
                        # (bias rides the ones-plane through the comb matmul)
                        nc.scalar.activation(
                            sg[:, :], pzm[:, :], SIG, scale=1.0 / WSC
                        )
                        sig_fi = sg[:, 0:CH]
                        sig_og = sg[:, CH : 2 * CH]
                        # tanh(z_g) = 2*sig(2 z_g) - 1  (stays @64-127)
                        nc.vector.tensor_scalar(
                            yp[Fl : 2 * Fl, :], sg[Fl : 2 * Fl, CH : 2 * CH],
                            2.0, -1.0, MULT, ADD,
                        )
                        # P1 = sig_i * tanh_g  (ins @64-127, out @0-63)
                        nc.vector.tensor_tensor(
                            yp[0:Fl, :], sg[Fl : 2 * Fl, 0:CH],
                            yp[Fl : 2 * Fl, :], MULT,
                        )
                        # P2 = sig_f * c (in place over sig_f)
                        nc.vector.tensor_tensor(
                            sg[0:Fl, 0:CH], sg[0:Fl, 0:CH],
                            ct[:, px0 : px0 + CH], MULT,
                        )
                        # c' = P1 + P2
                        nc.vector.tensor_tensor(
                            ct[:, px0 : px0 + CH], yp[0:Fl, :], sg[0:Fl, 0:CH], ADD
                        )
                        stash[oc] = sg

                    def phase_y(oc):
                        y0 = oc * RPC
                        px0 = oc * CH
                        sg = stash.pop(oc)
                        tc_t = gs.tile([Fl, CH], HDT, tag="tc_t",
                                       name=f"tc_t_{t}_{oc}")
                        # tanh(c) directly — same act table as sigmoid
                        nc.scalar.activation(tc_t[:, :], ct[:, px0 : px0 + CH], TANH)
                        # h = tanh_c * sig_o -> S0 (dup[64:128], rows y0+1..)
                        so_v = sg[0:Fl, CH : 2 * CH].rearrange(
                            "p (r c) -> p r c", c=W
                        )
                        tcv = tc_t[:, :].rearrange("p (r c) -> p r c", c=W)
                        nc.vector.tensor_tensor(
                            dup[Fl : 2 * Fl, y0 + 1 : y0 + 1 + RPC, 1 : 1 + W],
                            tcv, so_v, MULT,
                        )
                        src = dup[Fl : 2 * Fl, y0 + 1 : y0 + 1 + RPC, 1 : 1 + W]
                        if t < G.T - 1:
                            # cast h into the fp8 recurrent tile (S0 slot +
                            # S1 row-shifted slot) on the idle DMA engines;
                            # only gpsimd-initiated DMAs may cast
                            nc.gpsimd.dma_start(
                                dupq[Fl : 2 * Fl, y0 + 1 : y0 + 1 + RPC, 1 : 1 + W],
                                src,
                            )
                            nc.gpsimd.dma_start(
                                dupq[0:Fl, y0 : y0 + RPC, 1 : 1 + W], src
                            )
                        else:
                            # final step: bf16 shifted copies for the decoder
                            nc.vector.tensor_copy(
                                dup[0:Fl, y0 : y0 + RPC, 1 : 1 + W], src
                            )
                            nc.gpsimd.tensor_copy(
                                hx2[Fl : 2 * Fl, y0 : y0 + RPC, 1 : 1 + W], src
                            )
                            nc.gpsimd.tensor_copy(
                                hx2[0:Fl, y0 : y0 + RPC, 2 : 2 + W], src
                            )

                    for j in range(G.NOC + SKEW):
                        if j < G.NOC:
                            phase_x(j)
                        if j >= SKEW:
                            phase_y(j - SKEW)

            if debug_state:
                with tc.tile_pool(name="dbgp", bufs=1) as dbp:
                    dbf = dbp.tile([F, G.HP * G.WP], F32)
                    nc.vector.tensor_copy(dbf[:, :], dup[F : 2 * F, :, :].rearrange("p a b -> p (a b)"))
                    nc.sync.dma_start(dbg_h[:, :, :].rearrange("p a b -> p (a b)"), dbf[:, :])
                    nc.vector.tensor_copy(dbf[:, :], dup[0:F, :, :].rearrange("p a b -> p (a b)"))
                    nc.sync.dma_start(dbg_h1[:, :, :].rearrange("p a b -> p (a b)"), dbf[:, :])
                    nc.sync.dma_start(dbg_c[:, :], ct[:, :])

            # ---------------- decoder ----------------
            if phase in ("all", "dec", "dec0", "dec1"):
              with (
                tc.tile_pool(name="psd", bufs=4, space="PSUM") as psd,
                tc.tile_pool(name="ds", bufs=1) as dsp,
                tc.tile_pool(name="ds2", bufs=6) as ds2,
              ):
                # pred0 = sigmoid(conv(h, w_out) + b_out), M=1 matmuls
                wp0d = []
                for i in range(3):
                    tw = dsp.tile([2 * Fl, 1], HDT, tag=f"wp0d{i}")
                    nc.sync.dma_start(tw[:, :], d_p0dom[i])
                    wp0d.append(tw)
                wp0s = dsp.tile([2 * Fl, 1], HDT, tag="wp0s")
                nc.sync.dma_start(wp0s[:, :], d_p0hdom[:, :])
                wp0c = dsp.tile([Fl, 1], HDT, tag="wp0c")
                nc.sync.dma_start(wp0c[:, :], d_p0c11[:, :])

                wdec = []
                for k in range(9):
                    tw = dsp.tile([G.H, G.H], HDT, tag=f"wdec{k}")
                    nc.gpsimd.dma_start(tw[:, :], d_wdec[k])
                    wdec.append(tw)

                pb = dp.tile([G.H * G.W], F32)  # DRAM bounce for reshape

                nsub_all = (G.H // SUB)
                for s in range(nsub_all):
                    ys = s * SUB
                    pzp = psd.tile([128, N], F32, tag="pzp")
                    mm = []
                    for i, dx in enumerate((-1, 0, 1)):
                        mm.append(
                            (wp0d[i][:, :], dup[:, ys : ys + SUB, 1 + dx : 1 + dx + W])
                        )
                    mm.append(
                        (wp0s[:, :], hx2[:, ys + 1 : ys + 1 + SUB, 1 : 1 + W])
                    )
                    # tap (1,1) via the S1 copy: S1[r, c] = h[r, c-1], so
                    # reading (ys+1+j, 2+k) yields h[ys+1+j, k+1]
                    mm.append(
                        (wp0c[:, :], dup[0:Fl, ys + 1 : ys + 1 + SUB, 2 : 2 + W])
                    )
                    for i, (lhsT, rhs) in enumerate(mm):
                        nc.tensor.matmul(
                            pzp[0:1, :], lhsT, rhs,
                            start=(i == 0), stop=(i == len(mm) - 1),
                        )
                    p0s = ds2.tile([1, N], F32, tag="p0s")
                    nc.scalar.activation(p0s[:, :], pzp[0:1, :], SIG, bias=b_out_f)
                    nc.sync.dma_start(out[0:1, ys * W : (ys + SUB) * W], p0s[0:1, :])
                    nc.sync.dma_start(
                        pb[ys * W : (ys + SUB) * W].rearrange("(a b) -> a b", a=1),
                        p0s[0:1, :],
                    )

                if phase == "dec0":
                    nc.compile._noop if False else None
                predT = dsp.tile([G.H, WP], HDT, tag="predT")
                if phase not in ("dec0",):
                  nc.vector.memset(predT[:, :], 0.0)
                  nc.gpsimd.dma_start(
                    predT[:, 1 : 1 + W], pb[:].rearrange("(h w) -> h w", w=W)
                  )

                if phase == "dec1":
                    it_range = []
                elif phase == "dec0":
                    it_range = []
                else:
                    it_range = list(range(1, G.PRED))
                for k in it_range:
                    pzd = psd.tile([G.H, W], F32, tag="pzd")
                    for i, (dy, dx) in enumerate(TAPS):
                        nc.tensor.matmul(
                            pzd[:, :],
                            wdec[i][:, :],
                            predT[:, 1 + dx : 1 + dx + W],
                            start=(i == 0),
                            stop=(i == 8),
                        )
                    nc.scalar.activation(predT[:, 1 : 1 + W], pzd[:, :], SIG, bias=c0_f)
                    # casting DMA (bf16 -> f32 out) keeps the serial chain at
                    # one cross-engine hop per step: act -> next matmul
                    nc.gpsimd.dma_start(
                        out[k, :].rearrange("(h w) -> h w", w=W), predT[:, 1 : 1 + W]
                    )

    nc.compile()
    return nc


PROFILE = False          # set True (e.g. from test.py) to capture an NTFF trace
LAST_EXEC_NS = None
LAST_TRACE_DIR = None


def _run_full(inputs):
    import tempfile
    from concourse.bass_utils import run_bass_kernel_spmd

    global LAST_EXEC_NS, LAST_TRACE_DIR
    G = Geo(H, W, T, PRED)
    x = np.asarray(inputs["x"], np.float32)  # (B,T,H,W,1)
    packed, b_out_f, c0_f = pack_host(
        G,
        inputs["kernel"],
        inputs["rec_kernel"],
        inputs["bias"],
        inputs["w_out"],
        inputs["b_out"],
        inputs["w_proj"],
        inputs["b_proj"],
    )
    nc = build(G, b_out_f, c0_f)
    in_maps = []
    for b in range(B):
        m = dict(packed)
        m["xcol"] = make_xcol(G, x[b, :, :, :, 0])
        in_maps.append(m)
    if PROFILE:
        results, LAST_EXEC_NS = _timed_pjrt(nc, in_maps, B)
    else:
        res = run_bass_kernel_spmd(nc, in_maps, core_ids=list(range(B)))
        results = res.results
        LAST_EXEC_NS = res.exec_time_ns
    outs = np.stack([results[b]["out"] for b in range(B)], axis=0)
    return outs.reshape(B, PRED, H, W, 1).astype(np.float32)


def _timed_pjrt(nc, in_maps, n_cores, iters=5):
    """Mirror bass2jax.run_bass_via_pjrt's multi-core path but reuse one
    jitted executable and time warm invocations (device-blocking, no D2H)."""
    import time
    import jax
    import concourse.mybir as mybir
    from concourse import bass2jax
    from jax.sharding import Mesh, PartitionSpec
    from jax.experimental.shard_map import shard_map

    bass2jax.install_neuronx_cc_hook()
    partition_name = nc.partition_id_tensor.name if nc.partition_id_tensor else None

    in_names, out_names, out_avals, zero_outs = [], [], [], []
    for alloc in nc.m.functions[0].allocations:
        if not isinstance(alloc, mybir.MemoryLocationSet):
            continue
        name = alloc.memorylocations[0].name
        if alloc.kind == "ExternalInput":
            if name != partition_name:
                in_names.append(name)
        elif alloc.kind == "ExternalOutput":
            shape = tuple(alloc.tensor_shape)
            dtype = mybir.dt.np(alloc.dtype)
            out_names.append(name)
            out_avals.append(jax.core.ShapedArray(shape, dtype))
            zero_outs.append(np.zeros(shape, dtype))
    n_params = len(in_names)
    n_outs = len(out_avals)
    all_in_names = list(in_names) + list(out_names)
    if partition_name is not None:
        all_in_names.append(partition_name)

    donate = tuple(range(n_params, n_params + n_outs))

    def _body(*args):
        operands = list(args)
        if partition_name is not None:
            operands.append(bass2jax.partition_id_tensor())
        outs = bass2jax._bass_exec_p.bind(
            *operands,
            out_avals=tuple(out_avals),
            in_names=tuple(all_in_names),
            out_names=tuple(out_names),
            lowering_input_output_aliases=(),
            sim_require_finite=True,
            sim_require_nnan=True,
            nc=nc,
        )
        return tuple(outs)

    devices = jax.devices()[:n_cores]
    mesh = Mesh(np.asarray(devices), ("core",))
    in_specs = (PartitionSpec("core"),) * (n_params + n_outs)
    out_specs = (PartitionSpec("core"),) * n_outs
    sharded = jax.jit(
        shard_map(
            _body, mesh=mesh, in_specs=in_specs, out_specs=out_specs, check_rep=False
        ),
        donate_argnums=donate,
        keep_unused=True,
    )
    concat_in = [
        np.concatenate([np.asarray(in_maps[c][nm]) for c in range(n_cores)], axis=0)
        for nm in in_names
    ]

    def zeros():
        return [
            np.zeros((n_cores * z.shape[0], *z.shape[1:]), z.dtype) for z in zero_outs
        ]

    out_arrs = sharded(*concat_in, *zeros())  # compile + first run
    jax.block_until_ready(out_arrs)
    results = [
        {
            nm: np.asarray(out_arrs[i]).reshape(n_cores, *out_avals[i].shape)[c]
            for i, nm in enumerate(out_names)
        }
        for c in range(n_cores)
    ]

    sharding = jax.sharding.NamedSharding(mesh, PartitionSpec("core"))
    concat_in_dev = [jax.device_put(a, sharding) for a in concat_in]
    jax.block_until_ready(concat_in_dev)
    times = []
    for _ in range(iters):
        zs = [jax.device_put(z, sharding) for z in zeros()]
        jax.block_until_ready(zs)
        t0 = time.perf_counter()
        oa = sharded(*concat_in_dev, *zs)
        jax.block_until_ready(oa)
        times.append(time.perf_counter() - t0)
    best_ns = int(min(times) * 1e9)
    return results, best_ns


def kernel(**inputs) -> np.ndarray:
    return _run_full(inputs)

